# revision 1
# baseline (speedup 1.0000x reference)
"""Trainium2 Bass kernel for a 2-layer GAT + MLP (nn_MemoryGNN).

Strategy (8 NeuronCores, SPMD):
  - Destination-node partition: core k owns dst nodes [k*6250, (k+1)*6250).
  - Every core computes the FULL H1 = x @ [W1|U1|V1] table (x is broadcast by
    the host for free), so layer-1 message gathers are purely local.
  - Per-edge gathers use gpsimd.dma_gather from padded per-dst-tile slot
    tables (host-precomputed int16 index blobs).  Padding slots point at a
    dummy row whose attention-score column is -1e30, so exp() gives them zero
    weight with no masking ops.
  - Softmax is computed unnormalized (exp without segment-max subtraction;
    scores are O(10) so this is safe) and num/den partials are accumulated.
  - Layer 2 needs remote H2 rows: a chunked AllGather of the per-core H2
    shards overlaps with layer-1 compute.
  - Output rows are produced in a degree-sorted permuted order; the host
    applies the inverse permutation (free).
"""

import sys
import numpy as np

for _p in ("/opt/trn_rl_repo", "/root/.axon_site/_ro/trn_rl_repo"):
    if _p not in sys.path:
        sys.path.insert(0, _p)

import concourse.bass as bass
import concourse.bacc as bacc
import concourse.mybir as mybir
import concourse.tile as tile
from concourse import library_config
from concourse.bass_utils import run_bass_kernel_spmd

F32 = mybir.dt.float32
I16 = mybir.dt.int16
AF = mybir.ActivationFunctionType
OP = mybir.AluOpType
AX = mybir.AxisListType

NEG_SLOPE = 0.2


def make_cfg(N=50000, E=1000000, IN_DIM=256, HID=64, HEADS=4, OUT_DIM=128,
             NC=8, CHT=7, KCAP1=32, KCAP2=32):
    cfg = dict(N=N, E=E, IN_DIM=IN_DIM, HID=HID, HEADS=HEADS, OUT_DIM=OUT_DIM,
               NC=NC, CHT=CHT, KCAP1=KCAP1, KCAP2=KCAP2)
    cfg["SHARD"] = N // NC
    assert N % NC == 0
    TP = 128
    cfg["TP"] = TP
    NT = -(-cfg["SHARD"] // TP)
    cfg["NT"] = NT
    assert NT % CHT == 0, (NT, CHT)
    cfg["NCH"] = NT // CHT
    cfg["ROWS"] = NT * TP
    cfg["CHROWS"] = CHT * TP          # SH2 rows per AllGather chunk
    # layer-1 table: row n -> n + (n >= LO1); 2 dummy rows
    cfg["D1"] = IN_DIM + 2 * HEADS    # used row width (H1 | ssrc | sdst)
    cfg["W1R"] = -(-cfg["D1"] // 64) * 64
    cfg["LO1"] = (N // 2 + 63) // 64 * 64
    assert cfg["LO1"] + 1 <= 32767 and N - cfg["LO1"] + 1 <= 32767
    cfg["HX1_ROWS"] = N + 2
    # layer-2 table (chunk-major): [chunks 0..L-1 | dum | chunks L..NCH-1 | dum]
    cfg["D2"] = OUT_DIM + 2
    cfg["W2R"] = -(-cfg["D2"] // 64) * 64
    CH_ALL = cfg["CHROWS"] * NC       # global rows per chunk
    cfg["CH_ALL"] = CH_ALL
    LOCH = NC * cfg["ROWS"] // 2 // CH_ALL   # chunks in the lo half
    LOCH = max(1, min(cfg["NCH"] - 1, LOCH))
    cfg["LOCH"] = LOCH
    cfg["LO2ROWS"] = LOCH * CH_ALL
    assert cfg["LO2ROWS"] + 1 <= 32767
    assert (cfg["NCH"] - LOCH) * CH_ALL + 1 <= 32767
    cfg["HX2_ROWS"] = cfg["NCH"] * CH_ALL + 2
    return cfg


# ----------------------------------------------------------------- host prep

def _wrap16(flat):
    """flat int array (len divisible by 16) -> wrapped [128, n/16] int16."""
    w = flat.reshape(-1, 16).T.astype(np.int16)
    return np.tile(w, (8, 1))


def _pack_core(cfg, srcs_by_dst, row_of_src, lo_limit, dum_lo, dum_hi, kcap):
    """For one core: sort dsts by (lo,hi) counts, tile, build index blob.

    srcs_by_dst: list over local dst ids of arrays of table rows (already
    mapped through row_of_src).  Returns (perm, vtiles, blob_cols) where
    vtiles is a list per real tile of [(kl, kh), ...] sub-iterations and
    blob_cols the per-tile wrapped int16 column blocks (as arrays).
    """
    SHARD, TP, NT = cfg["SHARD"], cfg["TP"], cfg["NT"]
    lo_cnt = np.array([int((s < lo_limit).sum()) for s in srcs_by_dst])
    hi_cnt = np.array([len(s) for s in srcs_by_dst]) - lo_cnt
    order = np.lexsort((-hi_cnt, -lo_cnt))
    perm = np.full(NT * TP, -1, dtype=np.int64)
    perm[:SHARD] = order
    kl_t = np.zeros(NT, dtype=np.int64)
    kh_t = np.zeros(NT, dtype=np.int64)
    for t in range(NT):
        rows = perm[t * TP:(t + 1) * TP]
        real = rows[rows >= 0]
        if len(real):
            kl_t[t] = lo_cnt[real].max()
            kh_t[t] = hi_cnt[real].max()
    return perm, lo_cnt, hi_cnt, kl_t, kh_t


def _build_blobs(cfg, perm, srcs_by_dst, lo_limit, dum_lo, dum_hi,
                 kl_t, kh_t, kcap, dst_rows):
    """Build the per-core int16 index blob.

    Per tile layout: [dst-lo idx (8 cols) | dst-hi idx (8 cols) |
                      per-vtile (lo slots kl_v*8 | hi slots kh_v*8) ...]
    dst_rows: table row of each local dst (for the sdst gather).
    Returns (blob [128, C] int16, vtiles list, col offsets dict).
    """
    TP, NT = cfg["TP"], cfg["NT"]
    cols = []
    meta = []
    for t in range(NT):
        rows = perm[t * TP:(t + 1) * TP]
        # dst gathers (lo/hi split with additive dummy)
        dlo = np.full(TP, dum_lo, dtype=np.int64)
        dhi = np.full(TP, dum_hi - lo_limit, dtype=np.int64)
        for p, r in enumerate(rows):
            if r >= 0:
                dr = dst_rows[r]
                if dr < lo_limit:
                    dlo[p] = dr
                else:
                    dhi[p] = dr - lo_limit
        tile_cols = [_wrap16(dlo), _wrap16(dhi)]
        # slot tables
        lo_mat = np.full((TP, max(1, kl_t[t])), dum_lo, dtype=np.int64)
        hi_mat = np.full((TP, max(1, kh_t[t])), dum_hi - lo_limit, dtype=np.int64)
        for p, r in enumerate(rows):
            if r >= 0:
                s = srcs_by_dst[r]
                lo = s[s < lo_limit]
                hi = s[s >= lo_limit] - lo_limit
                lo_mat[p, :len(lo)] = lo
                hi_mat[p, :len(hi)] = hi
        # split into per-half vtiles of slot count <= kcap
        vt = []
        la, ha = int(kl_t[t]), int(kh_t[t])
        off = 0
        while off < la:
            kv = min(kcap, la - off)
            vt.append((0, kv))
            tile_cols.append(_wrap16(lo_mat[:, off:off + kv].T.reshape(-1)))
            off += kv
        off = 0
        while off < ha:
            kv = min(kcap, ha - off)
            vt.append((1, kv))
            tile_cols.append(_wrap16(hi_mat[:, off:off + kv].T.reshape(-1)))
            off += kv
        meta.append(vt)
        cols.append(np.concatenate(tile_cols, axis=1))
    blob = np.concatenate(cols, axis=1)
    return blob, meta


def ho_remaining(ha, hi_off):
    return hi_off < ha


def _prep(cfg, edge_index):
    """Host preprocessing. Returns per-core blobs + tile structure + perms."""
    N, NC, SHARD = cfg["N"], cfg["NC"], cfg["SHARD"]
    TP, NT, ROWS = cfg["TP"], cfg["NT"], cfg["ROWS"]
    src = np.concatenate([np.asarray(edge_index[0]), np.arange(N)]).astype(np.int64)
    dst = np.concatenate([np.asarray(edge_index[1]), np.arange(N)]).astype(np.int64)
    order = np.argsort(dst, kind="stable")
    src_s, dst_s = src[order], dst[order]
    deg = np.bincount(dst, minlength=N)
    rp = np.zeros(N + 1, dtype=np.int64)
    np.cumsum(deg, out=rp[1:])

    LO1 = cfg["LO1"]
    row1_of = lambda n: n + (n >= LO1)          # noqa: E731
    dum1_lo, dum1_hi = LO1, N + 1               # table rows (hi is global row)

    cores = []
    for k in range(NC):
        g0 = k * SHARD
        srcs_by_dst = [src_s[rp[g0 + r]:rp[g0 + r + 1]] for r in range(SHARD)]
        rows1 = [row1_of(s) for s in srcs_by_dst]
        perm1, lo1, hi1, kl1, kh1 = _pack_core(
            cfg, rows1, None, LO1, dum1_lo, dum1_hi, cfg["KCAP1"])
        cores.append(dict(srcs_by_dst=srcs_by_dst, rows1=rows1, perm1=perm1,
                          kl1=kl1, kh1=kh1))

    # uniform tile sizes across cores
    KL1 = np.max([c["kl1"] for c in cores], axis=0)
    KH1 = np.max([c["kh1"] for c in cores], axis=0)

    # HX2 chunk-major row of each global node (needs perm1 of its owner)
    pos1 = np.empty(N, dtype=np.int64)
    for k in range(NC):
        p = cores[k]["perm1"]
        for q in range(ROWS):
            if p[q] >= 0:
                pos1[k * SHARD + p[q]] = q
    CHROWS, CH_ALL = cfg["CHROWS"], cfg["CH_ALL"]
    c_of = pos1 // CHROWS
    r_of = pos1 % CHROWS
    owner = np.arange(N) // SHARD
    cm = c_of * CH_ALL + owner * CHROWS + r_of
    LO2 = cfg["LO2ROWS"]
    row2_of_node = cm + (cm >= LO2)             # dummy inserted at LO2
    dum2_lo = LO2
    dum2_hi = cfg["HX2_ROWS"] - 1

    for k in range(NC):
        c = cores[k]
        rows2 = [row2_of_node[s] for s in c["srcs_by_dst"]]
        perm2, lo2, hi2, kl2, kh2 = _pack_core(
            cfg, rows2, None, LO2 + 1, dum2_lo, dum2_hi, cfg["KCAP2"])
        c.update(rows2=rows2, perm2=perm2, kl2=kl2, kh2=kh2)

    KL2 = np.max([c["kl2"] for c in cores], axis=0)
    KH2 = np.max([c["kh2"] for c in cores], axis=0)

    # build blobs with the uniform sizes
    blobs1, blobs2 = [], []
    meta1 = meta2 = None
    for k in range(NC):
        c = cores[k]
        d1rows = np.array([row1_of(k * SHARD + r) for r in range(SHARD)])
        b1, m1 = _build_blobs(cfg, c["perm1"], c["rows1"], LO1 + 1, dum1_lo,
                              dum1_hi, KL1, KH1, cfg["KCAP1"], d1rows)
        d2rows = np.array([row2_of_node[k * SHARD + r] for r in range(SHARD)])
        b2, m2 = _build_blobs(cfg, c["perm2"], c["rows2"], LO2 + 1, dum2_lo,
                              dum2_hi, KL2, KH2, cfg["KCAP2"], d2rows)
        blobs1.append(b1)
        blobs2.append(b2)
        meta1, meta2 = m1, m2   # identical structure across cores
    return dict(cores=cores, blobs1=blobs1, blobs2=blobs2,
                vt1=meta1, vt2=meta2, KL1=KL1, KH1=KH1, KL2=KL2, KH2=KH2)


def _pack_consts(cfg, W1, a1_src, a1_dst, b1, W2, a2_src, a2_dst, b2,
                 Wm1, bm1, Wm2, bm2):
    IN_DIM, HID, HEADS, OUT_DIM = (cfg["IN_DIM"], cfg["HID"], cfg["HEADS"],
                                   cfg["OUT_DIM"])
    D1, D2, W2R = cfg["D1"], cfg["D2"], cfg["W2R"]
    U1 = np.einsum("khc,hc->kh", W1.reshape(IN_DIM, HEADS, HID), a1_src)
    V1 = np.einsum("khc,hc->kh", W1.reshape(IN_DIM, HEADS, HID), a1_dst)
    W1R = cfg["W1R"]
    W1X = np.zeros((IN_DIM, W1R), dtype=np.float32)
    W1X[:, :D1] = np.concatenate([W1, U1, V1], axis=1)
    u2 = W2 @ a2_src[0]
    v2 = W2 @ a2_dst[0]
    W2X = np.zeros((HEADS * HID, W2R), dtype=np.float32)
    W2X[:, :OUT_DIM] = W2
    W2X[:, OUT_DIM] = u2
    W2X[:, OUT_DIM + 1] = v2
    P = 128
    blocks = {}
    cols = 0
    def add(name, arr):
        nonlocal cols
        a = np.zeros((P, arr.shape[1]), dtype=np.float32)
        a[:arr.shape[0]] = arr
        blocks[name] = (cols, arr.shape[1])
        cols += arr.shape[1]
        return a
    parts = []
    parts.append(add("w1x0", W1X[0:P]))
    parts.append(add("w1x1", W1X[P:2 * P]))
    parts.append(add("w2x0", W2X[0:P]))
    parts.append(add("w2x1", W2X[P:2 * P]))
    parts.append(add("wm1", Wm1.astype(np.float32)))
    parts.append(add("wm2", Wm2.astype(np.float32)))
    parts.append(add("b1r", np.tile(b1.astype(np.float32), (P, 1))))
    parts.append(add("b2r", np.tile(b2.astype(np.float32), (P, 1))))
    parts.append(add("bm1r", np.tile(bm1.astype(np.float32), (P, 1))))
    parts.append(add("bm2r", np.tile(bm2.astype(np.float32), (P, 1))))
    parts.append(add("ident", np.eye(P, dtype=np.float32)))
    consts = np.concatenate(parts, axis=1)
    # dummy rows: [hx1_lo, hx1_hi, hx2_lo, hx2_hi] in a [4, W1R] array
    dums = np.zeros((4, cfg["W1R"]), dtype=np.float32)
    dums[0:2, IN_DIM:IN_DIM + HEADS] = -1e30        # ssrc1
    dums[2:4, OUT_DIM] = -1e30                      # ssrc2
    return consts, blocks, dums


# ------------------------------------------------------------- device build

def _build(cfg, prep, cblocks, CW, phase="full"):
    N, NC = cfg["N"], cfg["NC"]
    IN_DIM, HID, HEADS, OUT_DIM = (cfg["IN_DIM"], cfg["HID"], cfg["HEADS"],
                                   cfg["OUT_DIM"])
    TP, NT, ROWS, SHARD = cfg["TP"], cfg["NT"], cfg["ROWS"], cfg["SHARD"]
    D1, W1R, LO1 = cfg["D1"], cfg["W1R"], cfg["LO1"]
    D2, W2R, LO2 = cfg["D2"], cfg["W2R"], cfg["LO2ROWS"]
    CHT, NCH, CHROWS, CH_ALL = cfg["CHT"], cfg["NCH"], cfg["CHROWS"], cfg["CH_ALL"]
    HX1R, HX2R = cfg["HX1_ROWS"], cfg["HX2_ROWS"]
    KL1, KH1, KL2, KH2 = prep["KL1"], prep["KH1"], prep["KL2"], prep["KH2"]
    vt1, vt2 = prep["vt1"], prep["vt2"]
    C1 = prep["blobs1"][0].shape[1]
    C2 = prep["blobs2"][0].shape[1]
    NH2 = HEADS * HID

    nc = bacc.Bacc("TRN2", target_bir_lowering=False, debug=False,
                   num_devices=NC)
    xT = nc.dram_tensor("xT", [IN_DIM, N], F32, kind="ExternalInput")
    consts = nc.dram_tensor("consts", [128, CW], F32, kind="ExternalInput")
    dums = nc.dram_tensor("dums", [4, W1R], F32, kind="ExternalInput")
    idx1 = nc.dram_tensor("idx1", [128, C1], I16, kind="ExternalInput")
    idx2 = nc.dram_tensor("idx2", [128, C2], I16, kind="ExternalInput")
    out = nc.dram_tensor("out", [ROWS, OUT_DIM], F32, kind="ExternalOutput")
    dbg = nc.dram_tensor("dbg", [ROWS, W1R], F32, kind="ExternalOutput")

    HX1 = nc.dram_tensor("HX1", [HX1R, W1R], F32)
    HX2 = nc.dram_tensor("HX2", [HX2R, W2R], F32)
    SH2 = nc.dram_tensor("SH2", [ROWS, W2R], F32)

    hx1_lo = HX1[0:LO1 + 1, :]
    hx1_hi = HX1[LO1 + 1:HX1R, :]
    hx2_lo = HX2[0:LO2 + 1, :]
    hx2_hi = HX2[LO2 + 1:HX2R, :]

    P = 128

    with tile.TileContext(nc) as tc:
        nc.gpsimd.load_library(library_config.mlp)
        with tc.tile_pool(name="cp", bufs=1) as cp:
            cb = cp.tile([128, CW], F32, tag="consts")
            nc.sync.dma_start(cb[:, :], consts[:, :])

            def C(name):
                off, w = cblocks[name]
                return cb[:, off:off + w]

            # dummy rows (DRAM -> DRAM)
            nc.sync.dma_start(HX1[LO1:LO1 + 1, :], dums[0:1, :])
            nc.sync.dma_start(HX1[HX1R - 1:HX1R, :], dums[1:2, :])
            nc.sync.dma_start(HX2[LO2:LO2 + 1, :], dums[2:3, 0:W2R])
            nc.sync.dma_start(HX2[HX2R - 1:HX2R, :], dums[3:4, 0:W2R])

            # ---------------- P0: full HX1 table -----------------------
            SB = 512
            nsb = -(-N // SB) if cfg.get("P0", 1) else 0
            with (
                tc.tile_pool(name="p0", bufs=2) as p0,
                tc.tile_pool(name="p0ps", bufs=8, space="PSUM") as p0ps,
            ):
                for sb in range(nsb):
                    base = sb * SB
                    cnt = min(SB, N - base)
                    nq = -(-cnt // P)
                    xt0 = p0.tile([P, cnt], F32, tag="xt0")
                    xt1 = p0.tile([P, cnt], F32, tag="xt1")
                    nc.sync.dma_start(xt0[:, :], xT[0:P, base:base + cnt])
                    nc.sync.dma_start(xt1[:, :], xT[P:2 * P, base:base + cnt])
                    hx4 = p0.tile([P, nq * W1R], F32, tag="hx4")
                    for q in range(nq):
                        pb = min(P, cnt - q * P)
                        ps = p0ps.tile([P, W1R], F32, tag="p0ps")
                        nc.tensor.matmul(ps[0:pb, :], xt0[:, q * P:q * P + pb],
                                         C("w1x0"), start=True, stop=False)
                        nc.tensor.matmul(ps[0:pb, :], xt1[:, q * P:q * P + pb],
                                         C("w1x1"), start=False, stop=True)
                        nc.scalar.copy(hx4[0:pb, q * W1R:(q + 1) * W1R],
                                       ps[0:pb, :])
                    # write rows [base, base+cnt) -> HX1 (split at LO1)
                    hx4v = hx4[:, :].rearrange("p (q w) -> p q w", q=nq)
                    def wr(a, b):   # node range [a, b) within this superblock
                        if a >= b:
                            return
                        ra = base + a + (1 if base + a >= LO1 else 0)
                        dv = HX1[ra:ra + (b - a), :]
                        qa, pa = divmod(a, P)
                        qb, pb_ = divmod(b - 1, P)
                        if qa == qb:
                            nc.sync.dma_start(
                                dv, hx4v[pa:pb_ + 1, qa, :])
                        else:
                            n0 = P - pa
                            nc.sync.dma_start(dv[0:n0, :], hx4v[pa:P, qa, :])
                            off = n0
                            for q in range(qa + 1, qb):
                                nc.sync.dma_start(dv[off:off + P, :],
                                                  hx4v[0:P, q, :])
                                off += P
                            nc.sync.dma_start(dv[off:, :],
                                              hx4v[0:pb_ + 1, qb, :])
                    if base < LO1 < base + cnt:
                        wr(0, LO1 - base)
                        wr(LO1 - base, cnt)
                    else:
                        wr(0, cnt)

            if phase == "p0":
                nc.sync.dma_start(dbg[0:128, :], HX1[0:128, :])
                nc.sync.dma_start(dbg[128:256, :],
                                  HX1[LO1 + 1:LO1 + 129, :])
            # ---------------- L1 + H2 prep + chunked AllGather ----------
            with (
                tc.tile_pool(name="l1", bufs=2) as l1,
                tc.tile_pool(name="l1b", bufs=2) as l1b,
                tc.tile_pool(name="l1ps", bufs=2, space="PSUM") as l1ps,
            ):
                col = [0]

                def idx_tile(pool, blob, ncols, tag):
                    it = pool.tile([128, ncols], I16, tag=tag)
                    nc.sync.dma_start(it[:, :],
                                      blob[:, col[0]:col[0] + ncols])
                    col[0] += ncols
                    return it

                L1T = cfg.get("L1T", NT)
                L1S = cfg.get("L1S", 3)
                for t in (range(L1T) if phase != "p0" else range(0)):
                    # sdst gathers
                    itl = idx_tile(l1b, idx1, 8, "it_dl")
                    ith = idx_tile(l1b, idx1, 8, "it_dh")
                    if L1S >= 1:
                        sdl = l1b.tile([P, 64], F32, tag="sdl")
                        sdh = l1b.tile([P, 64], F32, tag="sdh")
                        nc.gpsimd.dma_gather(
                            sdl[:, :].rearrange("p (j w) -> p j w", j=1),
                            hx1_lo[:, IN_DIM:IN_DIM + 64], itl[:, :], P, P,
                            64, elem_step=W1R, single_packet=False)
                        nc.gpsimd.dma_gather(
                            sdh[:, :].rearrange("p (j w) -> p j w", j=1),
                            hx1_hi[:, IN_DIM:IN_DIM + 64], ith[:, :], P, P,
                            64, elem_step=W1R, single_packet=False)
                        sd4 = l1b.tile([P, HEADS], F32, tag="sd4")
                        nc.vector.tensor_tensor(
                            sd4[:, :], sdl[:, HEADS:2 * HEADS],
                            sdh[:, HEADS:2 * HEADS], op=OP.add)
                    if L1S >= 2:
                        num = l1b.tile([P, NH2], F32, tag="num")
                        den = l1b.tile([P, HEADS], F32, tag="den")
                    L1G = cfg.get("L1G", 99)
                    for v, (half, kv) in enumerate(vt1[t]):
                        itv = idx_tile(l1b, idx1, kv * 8, "it_sl")
                        if v >= L1G:
                            nc.sync.dma_start(dbg[0:128, 0:kv * 8].bitcast(I16),
                                              itv[:, :]) if False else None
                            continue
                        hg = l1.tile([P, kv * W1R], F32, tag="hg")
                        nc.gpsimd.dma_gather(
                            hg[:, :].rearrange("p (j w) -> p j w", j=kv),
                            (hx1_lo if half == 0 else hx1_hi)[:, :],
                            itv[:, :], P * kv, P * kv, W1R,
                            single_packet=False)
                        if L1S == 0:
                            if t == 0 and v == 0:
                                nc.sync.dma_start(
                                    dbg[0:P, 0:min(W1R, kv * W1R)],
                                    hg[:, 0:min(W1R, kv * W1R)])
                            continue
                        hgv = hg[:, :].rearrange("p (j w) -> p j w", j=kv)
                        s = l1b.tile([P, kv * HEADS], F32, tag="s")
                        sv = s[:, :].rearrange("p (j h) -> p j h", j=kv)
                        nc.vector.tensor_tensor(
                            sv, hgv[:, :, IN_DIM:IN_DIM + HEADS],
                            sd4[:, :].unsqueeze(1).broadcast_to(
                                [P, kv, HEADS]), op=OP.add)
                        s2t = l1b.tile([P, kv * HEADS], F32, tag="s2t")
                        nc.vector.tensor_scalar_mul(s2t[:, :], s[:, :],
                                                    NEG_SLOPE)
                        w0 = l1b.tile([P, kv * HEADS], F32, tag="w0")
                        nc.vector.tensor_tensor(w0[:, :], s[:, :], s2t[:, :],
                                                op=OP.max)
                        w = l1b.tile([P, kv * HEADS], F32, tag="w")
                        nc.scalar.activation(w[:, :], w0[:, :], AF.Exp)
                        if L1S == 1:
                            if t == 0 and v == 0:
                                nc.sync.dma_start(dbg[0:P, 0:kv * HEADS],
                                                  w[:, :])
                            continue
                        wv = w[:, :].rearrange("p (j h) -> p j h", j=kv)
                        if v == 0:
                            dv = den[:, :]
                        else:
                            denv = l1b.tile([P, HEADS], F32, tag="denv")
                            dv = denv[:, :]
                        nc.vector.tensor_reduce(
                            dv, w[:, :].rearrange("p (j h) -> p h j", j=kv),
                            axis=AX.X, op=OP.add)
                        if v > 0:
                            nc.vector.tensor_tensor(den[:, :], den[:, :], dv,
                                                    op=OP.add)
                        tmp = l1.tile([P, kv * NH2], F32, tag="tmp")
                        tmpv = tmp[:, :].rearrange(
                            "p (j h c) -> p j h c", j=kv, h=HEADS)
                        nc.vector.tensor_tensor(
                            tmpv,
                            hgv[:, :, 0:IN_DIM].rearrange(
                                "p j (h c) -> p j h c", h=HEADS),
                            wv.unsqueeze(3).broadcast_to([P, kv, HEADS, HID]),
                            op=OP.mult)
                        if v == 0:
                            nv = num[:, :]
                        else:
                            numv = l1b.tile([P, NH2], F32, tag="numv")
                            nv = numv[:, :]
                        nc.vector.tensor_reduce(
                            nv, tmp[:, :].rearrange("p (j c) -> p c j", j=kv),
                            axis=AX.X, op=OP.add)
                        if v > 0:
                            nc.vector.tensor_tensor(num[:, :], num[:, :], nv,
                                                    op=OP.add)
                    if L1S < 3:
                        if L1S == 2 and t == 0:
                            nc.sync.dma_start(dbg[0:P, 0:NH2], num[:, :])
                            nc.sync.dma_start(dbg[0:P, NH2:NH2 + HEADS],
                                              den[:, :])
                        continue
                    dinv = l1b.tile([P, HEADS], F32, tag="dinv")
                    nc.vector.tensor_scalar_max(dinv[:, :], den[:, :], 1e-6)
                    nc.vector.reciprocal(dinv[:, :], dinv[:, :])
                    o = l1b.tile([P, NH2], F32, tag="o")
                    nc.vector.tensor_tensor(
                        o[:, :].rearrange("p (h c) -> p h c", h=HEADS),
                        num[:, :].rearrange("p (h c) -> p h c", h=HEADS),
                        dinv[:, :].unsqueeze(2).broadcast_to([P, HEADS, HID]),
                        op=OP.mult)
                    nc.vector.tensor_tensor(o[:, :], o[:, :], C("b1r"),
                                            op=OP.add)
                    # elu
                    m0 = l1b.tile([P, NH2], F32, tag="m0")
                    nc.vector.tensor_scalar_min(m0[:, :], o[:, :], 0.0)
                    em = l1b.tile([P, NH2], F32, tag="em")
                    nc.scalar.activation(em[:, :], m0[:, :], AF.Exp)
                    p1 = l1b.tile([P, NH2], F32, tag="p1")
                    nc.vector.tensor_scalar(p1[:, :], o[:, :], 0.0, -1.0,
                                            op0=OP.max, op1=OP.add)
                    eo = l1b.tile([P, NH2], F32, tag="eo")
                    nc.vector.tensor_tensor(eo[:, :], em[:, :], p1[:, :],
                                            op=OP.add)
                    # transpose + H2 matmul
                    o1T = l1b.tile([P, NH2], F32, tag="o1T")
                    for cchunk in range(NH2 // P):
                        pt = l1ps.tile([P, P], F32, tag="pt")
                        nc.tensor.transpose(
                            pt[:, :], eo[:, cchunk * P:(cchunk + 1) * P],
                            C("ident"))
                        nc.scalar.copy(o1T[:, cchunk * P:(cchunk + 1) * P],
                                       pt[:, :])
                    h2p = l1ps.tile([P, W2R], F32, tag="h2p")
                    nc.tensor.matmul(h2p[:, :], o1T[:, 0:P], C("w2x0"),
                                     start=True, stop=False)
                    nc.tensor.matmul(h2p[:, :], o1T[:, P:2 * P], C("w2x1"),
                                     start=False, stop=True)
                    sh2 = l1b.tile([P, W2R], F32, tag="sh2")
                    nc.scalar.copy(sh2[:, :], h2p[:, :])
                    nc.sync.dma_start(SH2[t * P:(t + 1) * P, :], sh2[:, :])

                    if (t + 1) % CHT == 0 and phase not in ("l1",):
                        c = t // CHT
                        base = c * CH_ALL + (1 if c >= cfg["LOCH"] else 0)
                        nc.gpsimd.collective_compute(
                            "AllGather", OP.bypass,
                            replica_groups=[list(range(NC))],
                            ins=[SH2[c * CHROWS:(c + 1) * CHROWS, :].opt()],
                            outs=[HX2[base:base + CH_ALL, :].opt()],
                        )

            if phase in ("l1", "ag"):
                for _t in range(NT):
                    nc.sync.dma_start(
                        dbg[_t * P:(_t + 1) * P, 0:W2R],
                        SH2[_t * P:(_t + 1) * P, :])
            # ---------------- L2 + MLP + normalize ----------------------
            with (
                tc.tile_pool(name="l2", bufs=2) as l2,
                tc.tile_pool(name="l2b", bufs=2) as l2b,
                tc.tile_pool(name="l2ps", bufs=2, space="PSUM") as l2ps,
            ):
                col2 = [0]

                def idx_tile2(ncols, tag):
                    it = l2b.tile([128, ncols], I16, tag=tag)
                    nc.sync.dma_start(it[:, :],
                                      idx2[:, col2[0]:col2[0] + ncols])
                    col2[0] += ncols
                    return it

                for t in (range(NT) if phase == "full" else range(0)):
                    itl = idx_tile2(8, "it_dl")
                    ith = idx_tile2(8, "it_dh")
                    sdl = l2b.tile([P, 64], F32, tag="sdl")
                    sdh = l2b.tile([P, 64], F32, tag="sdh")
                    nc.gpsimd.dma_gather(
                        sdl[:, :].rearrange("p (j w) -> p j w", j=1),
                        hx2_lo[:, OUT_DIM:OUT_DIM + 64], itl[:, :], P, P, 64,
                        elem_step=W2R, single_packet=False)
                    nc.gpsimd.dma_gather(
                        sdh[:, :].rearrange("p (j w) -> p j w", j=1),
                        hx2_hi[:, OUT_DIM:OUT_DIM + 64], ith[:, :], P, P, 64,
                        elem_step=W2R, single_packet=False)
                    sd1 = l2b.tile([P, 1], F32, tag="sd1")
                    nc.vector.tensor_tensor(sd1[:, :], sdl[:, 1:2],
                                            sdh[:, 1:2], op=OP.add)

                    num = l2b.tile([P, OUT_DIM], F32, tag="num")
                    den = l2b.tile([P, 1], F32, tag="den")
                    for v, (half, kv) in enumerate(vt2[t]):
                        hg = l2.tile([P, kv * W2R], F32, tag="hg")
                        itv = idx_tile2(kv * 8, "it_sl")
                        nc.gpsimd.dma_gather(
                            hg[:, :].rearrange("p (j w) -> p j w", j=kv),
                            (hx2_lo if half == 0 else hx2_hi)[:, :],
                            itv[:, :], P * kv, P * kv, W2R,
                            single_packet=False)
                        hgv = hg[:, :].rearrange("p (j w) -> p j w", j=kv)
                        s = l2b.tile([P, kv], F32, tag="s")
                        nc.vector.tensor_tensor(
                            s[:, :].unsqueeze(2),
                            hgv[:, :, OUT_DIM:OUT_DIM + 1],
                            sd1[:, :].unsqueeze(1).broadcast_to([P, kv, 1]),
                            op=OP.add)
                        s2t = l2b.tile([P, kv], F32, tag="s2t")
                        nc.vector.tensor_scalar_mul(s2t[:, :], s[:, :],
                                                    NEG_SLOPE)
                        w0 = l2b.tile([P, kv], F32, tag="w0")
                        nc.vector.tensor_tensor(w0[:, :], s[:, :], s2t[:, :],
                                                op=OP.max)
                        w = l2b.tile([P, kv], F32, tag="w")
                        nc.scalar.activation(w[:, :], w0[:, :], AF.Exp)
                        if v == 0:
                            dv = den[:, :]
                        else:
                            denv = l2b.tile([P, 1], F32, tag="denv")
                            dv = denv[:, :]
                        nc.vector.tensor_reduce(dv, w[:, :], axis=AX.X,
                                                op=OP.add)
                        if v > 0:
                            nc.vector.tensor_tensor(den[:, :], den[:, :], dv,
                                                    op=OP.add)
                        tmp = l2.tile([P, kv * OUT_DIM], F32, tag="tmp")
                        nc.vector.tensor_tensor(
                            tmp[:, :].rearrange("p (j c) -> p j c", j=kv),
                            hgv[:, :, 0:OUT_DIM],
                            w[:, :].unsqueeze(2).broadcast_to(
                                [P, kv, OUT_DIM]),
                            op=OP.mult)
                        if v == 0:
                            nv = num[:, :]
                        else:
                            numv = l2b.tile([P, OUT_DIM], F32, tag="numv")
                            nv = numv[:, :]
                        nc.vector.tensor_reduce(
                            nv, tmp[:, :].rearrange("p (j c) -> p c j", j=kv),
                            axis=AX.X, op=OP.add)
                        if v > 0:
                            nc.vector.tensor_tensor(num[:, :], num[:, :], nv,
                                                    op=OP.add)
                    dinv = l2b.tile([P, 1], F32, tag="dinv")
                    nc.vector.tensor_scalar_max(dinv[:, :], den[:, :], 1e-6)
                    nc.vector.reciprocal(dinv[:, :], dinv[:, :])
                    o2 = l2b.tile([P, OUT_DIM], F32, tag="o2")
                    nc.vector.tensor_scalar(o2[:, :], num[:, :], dinv[:, 0:1],
                                            None, op0=OP.mult)
                    nc.vector.tensor_tensor(o2[:, :], o2[:, :], C("b2r"),
                                            op=OP.add)
                    # MLP
                    pt2 = l2ps.tile([P, P], F32, tag="pt2")
                    nc.tensor.transpose(pt2[:, :], o2[:, :], C("ident"))
                    o2T = l2b.tile([P, P], F32, tag="o2T")
                    nc.scalar.copy(o2T[:, :], pt2[:, :])
                    h3p = l2ps.tile([P, HID], F32, tag="h3p")
                    nc.tensor.matmul(h3p[:, :], o2T[:, :], C("wm1"),
                                     start=True, stop=True)
                    h3 = l2b.tile([P, HID], F32, tag="h3")
                    nc.vector.tensor_tensor(h3[:, :], h3p[:, :], C("bm1r"),
                                            op=OP.add)
                    nc.scalar.activation(h3[:, :], h3[:, :], AF.Relu)
                    pt3 = l2ps.tile([HID, P], F32, tag="pt3")
                    nc.tensor.transpose(pt3[:, :], h3[:, :], C("ident"))
                    h3T = l2b.tile([HID, P], F32, tag="h3T")
                    nc.scalar.copy(h3T[:, :], pt3[:, :])
                    h4p = l2ps.tile([P, OUT_DIM], F32, tag="h4p")
                    nc.tensor.matmul(h4p[:, :], h3T[0:HID, :],
                                     C("wm2")[0:HID, :], start=True, stop=True)
                    h4 = l2b.tile([P, OUT_DIM], F32, tag="h4")
                    nc.vector.tensor_tensor(h4[:, :], h4p[:, :], C("bm2r"),
                                            op=OP.add)
                    hsq = l2b.tile([P, OUT_DIM], F32, tag="hsq")
                    nc.scalar.activation(hsq[:, :], h4[:, :], AF.Square)
                    n2 = l2b.tile([P, 1], F32, tag="n2")
                    nc.vector.tensor_reduce(n2[:, :], hsq[:, :], axis=AX.X,
                                            op=OP.add)
                    nin = l2b.tile([P, 1], F32, tag="nin")
                    nc.vector.tensor_scalar_max(nin[:, :], n2[:, :], 1e-12)
                    nc.scalar.activation(nin[:, :], nin[:, :], AF.Sqrt)
                    nc.vector.reciprocal(nin[:, :], nin[:, :])
                    of = l2b.tile([P, OUT_DIM], F32, tag="of")
                    nc.vector.tensor_scalar(of[:, :], h4[:, :], nin[:, 0:1],
                                            None, op0=OP.mult)
                    nc.sync.dma_start(out[t * P:(t + 1) * P, :], of[:, :])

    nc.compile()
    return nc


# ------------------------------------------------------------------ driver

def run(cfg, inputs, trace=False, phase="full"):
    x = np.asarray(inputs["x"], dtype=np.float32)
    edge_index = np.asarray(inputs["edge_index"])
    prep = _prep(cfg, edge_index)
    consts, cblocks, dums = _pack_consts(
        cfg, *[np.asarray(inputs[k], dtype=np.float32) for k in
               ("W1", "a1_src", "a1_dst", "b1", "W2", "a2_src", "a2_dst",
                "b2", "Wm1", "bm1", "Wm2", "bm2")])
    nc = _build(cfg, prep, cblocks, consts.shape[1], phase=phase)
    xT = np.ascontiguousarray(x.T)
    in_maps = []
    for k in range(cfg["NC"]):
        in_maps.append({
            "xT": xT, "consts": consts, "dums": dums,
            "idx1": np.ascontiguousarray(prep["blobs1"][k]),
            "idx2": np.ascontiguousarray(prep["blobs2"][k]),
        })
    res = run_bass_kernel_spmd(nc, in_maps, list(range(cfg["NC"])),
                               trace=trace)
    N, NC, SHARD = cfg["N"], cfg["NC"], cfg["SHARD"]
    full = np.zeros((N, cfg["OUT_DIM"]), dtype=np.float32)
    for k in range(NC):
        o = res.results[k]["out"]
        perm2 = prep["cores"][k]["perm2"]
        for r in range(cfg["ROWS"]):
            if perm2[r] >= 0:
                full[k * SHARD + perm2[r]] = o[r]
    return full, res


def kernel(**inputs):
    cfg = make_cfg()
    full, _ = run(cfg, inputs, trace=False)
    return full

